# revision 23
# baseline (speedup 1.0000x reference)
"""Causal self-attention Trainium2 kernel (8 NeuronCores, SPMD).

Sharding: 8 cores = 4 batches x 2 head-groups. Each core computes, for its
(batch b, head-group g): Q/K/V projections restricted to g's 8 heads
(column-parallel), causal attention for those heads, and the partial output
projection ctx_g @ Wo[g rows] (row-parallel). Host sums the two partials per
batch and adds the bias terms (bv @ Wo + bo).

All matmuls run in bf16 with fp32 PSUM accumulation. Attention uses the
transposed-scores orientation: scoresT[k, q] tiles are exp'd in place and fed
directly as the moving operand of the PV matmul (no PE transposes, all
matmuls N=512).

Pipeline (single PE stream, software-pipelined):
  P0   QK projection for head 0, chunk-wise over the contraction as the xT
       chunk DMAs land (PE starts ~4us into the kernel instead of ~30us).
  P1   V = x @ Wv (dense, 8 PSUM banks).
  S_h  (h = 0..7) attention for head h interleaved with the QK projection of
       head h+1: the scalar-engine exp stream hides under projection matmuls.
  P4   out = ctx @ Wo, evicted to bf16, host adds the two group partials.

Softmax details: no max subtraction (scores ~N(0,1); exp cannot overflow),
1/sqrt(dh) folded into the exp activation, causal masking via additive -1e30
mask slices (one ramp constant) applied on the DVE before a batched exp
(one activation instruction per two score tiles / PSUM banks). The
normalizer is accumulated on the DVE, partition-reduced AND broadcast in one
gpsimd partition_all_reduce, inverted with the fast approximate reciprocal,
and applied on the PV eviction multiply.
"""

import sys

sys.path.insert(0, "/opt/trn_rl_repo")

from contextlib import ExitStack

import numpy as np

import concourse.bass as bass
import concourse.tile as tile
from concourse import bass_isa, mybir
from concourse.bass_utils import run_bass_kernel_spmd

BF16 = mybir.dt.bfloat16
F32 = mybir.dt.float32
NP_BF16 = mybir.dt.np(BF16)

# Problem constants (hardcoded per contract).
B = 4          # batch
S = 2048       # sequence length
DM = 2048      # d_model
H = 16         # total heads
HD = 128       # head dim
G = 2          # head groups (tensor parallel degree)
NHL = H // G   # local heads per core
DHL = NHL * HD # local head dims
NCORES = 8
P = 128        # partitions
FD = 512       # matmul moving free dim (one PSUM bank of f32)
SCALE = 1.0 / float(np.sqrt(HD))
MASK_VAL = -1e30

# Opcodes whose walrus lowering handles multi-wait sync itself (or that we
# must not touch). Everything else gets its waits normalized to <= 1.
_WAIT_EXEMPT = {
    "NoOp",
    "EventSemaphore",
    "UnconditionalBranch",
    "RegisterMove",
    "TileRelease",
}


def _fix_sync_waits(nc, max_waits=1):
    """Hoist extra sync-waits onto single-wait NoOps on the issuing engine.

    Several walrus instruction encodings (PSEUDO_DMA_DIRECT2D, S3_LW, CTRL_NO,
    ...) have a single sync-wait slot and fail codegen with "Too many sync
    wait commands" when Tile attaches more. A NoOp on the same engine
    immediately before the instruction performs the extra wait at the
    sequencer, which is semantically identical.
    """
    f = nc.m.functions[0]
    fixed = 0

    def walk(blocks):
        nonlocal fixed
        for b in blocks:
            il = b.instructions
            i = 0
            while i < len(il):
                inst = il[i]
                si = getattr(inst, "sync_info", None)
                ow = list(si.on_wait) if si is not None and si.on_wait else []
                if inst.opcode not in _WAIT_EXEMPT and len(ow) > max_waits:
                    keep = ow[len(ow) - max_waits :]
                    extra = ow[: len(ow) - max_waits]
                    for j, w in enumerate(extra):
                        nop = mybir.InstNoOp(
                            name=f"{inst.name}_waitfix{j}",
                            engine=inst.engine,
                            ins=[],
                            outs=[],
                            bass_nofuse=True,
                            sync_info=mybir.SyncInfo(on_wait=[w], on_update=[]),
                        )
                        il.insert(i, nop)
                        i += 1
                    inst.sync_info = mybir.SyncInfo(
                        on_wait=keep,
                        on_update=list(si.on_update) if si.on_update else [],
                    )
                    fixed += 1
                i += 1
            walk(getattr(b, "blocks", []) or [])

    walk(f.blocks)
    return fixed


def _bcast_ap(ap, nparts):
    """Partition-broadcast view of a single-partition AP."""
    return bass.AP(
        tensor=ap.tensor, offset=ap.offset, ap=[[0, nparts]] + list(ap.ap[1:])
    )


def build_nc(seq=S, dm=DM, nhl=NHL, fix_waits=True):
    """Build the single-core Bass program (same program for all 8 cores)."""
    dhl = nhl * P
    nkc = dm // P    # contraction chunks for projections
    nst = seq // P   # seq (k-) tiles
    nqb = seq // FD  # 512-wide q blocks
    ktpb = FD // P   # k-tiles per q block (4)

    nc = bass.Bass()
    # All inputs are pre-arranged on the host into SBUF-friendly layouts so
    # every DMA is contiguous per partition line.
    xT_d = nc.dram_tensor("xT", [P, nkc, seq], BF16, kind="ExternalInput")
    wq_d = nc.dram_tensor("wq", [nhl, P, nkc, P], BF16, kind="ExternalInput")
    wk_d = nc.dram_tensor("wk", [nhl, P, nkc, P], BF16, kind="ExternalInput")
    wv_d = nc.dram_tensor("wv", [P, nkc, dhl], BF16, kind="ExternalInput")
    wo_d = nc.dram_tensor("wo", [P, dhl // P, dm], BF16, kind="ExternalInput")
    bqk_d = nc.dram_tensor("bqk", [P, 2, nhl], F32, kind="ExternalInput")
    out_d = nc.dram_tensor("out", [seq, dm], BF16, kind="ExternalOutput")

    with tile.TileContext(nc) as tc:
        # SBUF pools are LIFO stacks per side. Left stack, bottom to top:
        # consts, ctxT, vpool, qkpool (live to the end), then xpool (popped
        # mid-slots). Right stack: spool (strips, live to the end), wvpool
        # (popped after P1), then apool/zpool (slots onward), wopool (from
        # mid-slots, in xT's freed budget), opool (P4).
        es_top = ExitStack()
        consts = es_top.enter_context(tc.tile_pool(name="consts", bufs=1))
        ctxTpool = es_top.enter_context(tc.tile_pool(name="ctxTpool", bufs=1))
        ctxT = ctxTpool.tile([P, nhl, seq], BF16)
        vpool = es_top.enter_context(tc.tile_pool(name="vpool", bufs=1))
        V = vpool.tile([P, nst, dhl], BF16)
        qkpool = es_top.enter_context(tc.tile_pool(name="qkpool", bufs=2))
        QTs, KTs = {}, {}
        bqk_sb = consts.tile([P, 2, nhl], F32)
        ones_sb = consts.tile([P, 1], F32)
        nc.vector.memset(ones_sb[:, :], 1.0)
        # Causal ramp mask: rmask[p, c] = 0 if c >= p + 3*P else MASK_VAL.
        # Slice [:, (3-j)*P : (3-j)*P + FD] masks the transposed-score tile
        # whose k-tile sits j tiles past the q-block start (keep q >= k).
        rmask = consts.tile([P, FD + 3 * P], F32)
        nc.gpsimd.memset(rmask[:, :], 0.0)
        nc.gpsimd.affine_select(
            out=rmask[:, :],
            in_=rmask[:, :],
            compare_op=mybir.AluOpType.is_ge,
            fill=MASK_VAL,
            base=-3 * P,
            pattern=[[1, FD + 3 * P]],
            channel_multiplier=-1,
        )

        def mask_sl(j):
            return rmask[:, (3 - j) * P : (3 - j) * P + FD]

        nc.gpsimd.dma_start(out=bqk_sb[:, :, :], in_=bqk_d[:, :, :])

        # Strips for the per-head QK projections (right stack, rotates).
        es_strip = ExitStack()
        spool = es_strip.enter_context(
            tc.tile_pool(name="spool", bufs=2, side="right")
        )
        strips = {}

        def load_strip(h):
            wqs = spool.tile([P, nkc, P], BF16, tag="wqs", name=f"wqs{h}")
            wks = spool.tile([P, nkc, P], BF16, tag="wks", name=f"wks{h}")
            nc.gpsimd.dma_start(out=wqs[:, :, :], in_=wq_d[h, :, :, :])
            nc.gpsimd.dma_start(out=wks[:, :, :], in_=wk_d[h, :, :, :])
            strips[h] = (wqs, wks)

        load_strip(0)

        # xT streamed chunk-wise; head 0's QK projection chases the DMAs.
        es_x = ExitStack()
        xpool = es_x.enter_context(tc.tile_pool(name="xpool", bufs=1))
        xT = xpool.tile([P, nkc, seq], BF16)
        for i in range(nkc):
            nc.gpsimd.dma_start(out=xT[:, i : i + 1, :], in_=xT_d[:, i : i + 1, :])

        load_strip(1)

        es_wv = ExitStack()
        wvpool = es_wv.enter_context(
            tc.tile_pool(name="wvpool", bufs=1, side="right")
        )
        wv_sb = wvpool.tile([P, nkc, dhl], BF16)
        for i in range(0, nkc, 2):
            nc.gpsimd.dma_start(
                out=wv_sb[:, i : i + 2, :], in_=wv_d[:, i : i + 2, :]
            )

        # ---------------- P0: head-0 QK projection, chunk-wise ---------------
        es_p0 = ExitStack()
        p0 = es_p0.enter_context(tc.tile_pool(name="p0psum", bufs=1, space="PSUM"))
        qg = p0.tile([P, nqb, FD], F32, tag="qg")
        kg = p0.tile([P, nqb, FD], F32, tag="kg")
        wqs0, wks0 = strips[0]
        for c in range(nkc):
            for j in range(nqb):
                nc.tensor.matmul(
                    qg[:, j, :],
                    wqs0[:, c, :],
                    xT[:, c, j * FD : (j + 1) * FD],
                    start=(c == 0),
                    stop=(c == nkc - 1),
                )
            for j in range(nqb):
                nc.tensor.matmul(
                    kg[:, j, :],
                    wks0[:, c, :],
                    xT[:, c, j * FD : (j + 1) * FD],
                    start=(c == 0),
                    stop=(c == nkc - 1),
                )
        QTs[0] = qkpool.tile([P, nqb, FD], BF16, tag="QT", name="QT0")
        KTs[0] = qkpool.tile([P, nqb, FD], BF16, tag="KT", name="KT0")
        nc.scalar.activation(
            QTs[0][:, :, :],
            qg[:, :, :],
            mybir.ActivationFunctionType.Identity,
            bias=bqk_sb[:, 0, 0:1],
        )
        nc.scalar.activation(
            KTs[0][:, :, :],
            kg[:, :, :],
            mybir.ActivationFunctionType.Identity,
            bias=bqk_sb[:, 1, 0:1],
        )
        strips.pop(0)
        es_p0.close()

        # ---------------- P1: V = x @ Wv  ([seq, dhl] layout) ----------------
        es_pv0 = ExitStack()
        pv0 = es_pv0.enter_context(tc.tile_pool(name="pv0psum", bufs=8, space="PSUM"))
        for st in range(nst):
            for dc in range(dhl // FD):
                ps = pv0.tile([P, FD], F32, tag="v", bufs=8)
                for c in range(nkc):
                    nc.tensor.matmul(
                        ps[:, :],
                        xT[:, c, st * P : (st + 1) * P],
                        wv_sb[:, c, dc * FD : (dc + 1) * FD],
                        start=(c == 0),
                        stop=(c == nkc - 1),
                    )
                nc.vector.tensor_copy(V[:, st, dc * FD : (dc + 1) * FD], ps[:, :])
        es_pv0.close()
        es_wv.close()

        # ---------------- attention slots (+ interleaved QK projection) ------
        es_az = ExitStack()
        apool = es_az.enter_context(tc.tile_pool(name="apool", bufs=2, side="right"))
        zpool = es_az.enter_context(tc.tile_pool(name="zpool", bufs=2, side="right"))
        es_attn = ExitStack()
        spsum = es_attn.enter_context(tc.tile_pool(name="spsum", bufs=2, space="PSUM"))
        cpsum = es_attn.enter_context(tc.tile_pool(name="cpsum", bufs=2, space="PSUM"))
        jpsum = es_attn.enter_context(tc.tile_pool(name="jpsum", bufs=2, space="PSUM"))
        dpool = es_attn.enter_context(tc.tile_pool(name="dpool", bufs=2, space="DRAM"))

        state = {}

        def stage_scores(h, qb):
            """ScoresT tiles for (h, qb): matmul 2-bank groups, mask, exp."""
            kmax = (qb + 1) * ktpb
            exp_sb = apool.tile([P, nst, FD], BF16, tag="exp", name=f"exp{h}_{qb}")
            acc = zpool.tile([P, FD], F32, tag="acc", name=f"acc{h}_{qb}")
            for g in range(kmax // 2):
                sg = spsum.tile([P, 2, FD], F32, tag="sg", bufs=2, name=f"sg{g}")
                for i in range(2):
                    kt = 2 * g + i
                    nc.tensor.matmul(
                        sg[:, i, :],
                        KTs[h][:, kt // ktpb, (kt % ktpb) * P : (kt % ktpb + 1) * P],
                        QTs[h][:, qb, :],
                        start=True,
                        stop=True,
                    )
                    j = kt - ktpb * qb
                    if j >= 0:
                        # diagonal-adjacent tile: keep q >= k
                        nc.vector.tensor_add(
                            sg[:, i, :], sg[:, i, :], mask_sl(j)
                        )
                nc.scalar.activation(
                    exp_sb[:, 2 * g : 2 * g + 2, :],
                    sg[:, :, :],
                    mybir.ActivationFunctionType.Exp,
                    scale=SCALE,
                )
                # normalizer accumulation on the DVE
                if g == 0:
                    nc.vector.tensor_add(
                        acc[:, :], exp_sb[:, 0, :], exp_sb[:, 1, :]
                    )
                else:
                    nc.vector.tensor_add(acc[:, :], acc[:, :], exp_sb[:, 2 * g, :])
                    nc.vector.tensor_add(
                        acc[:, :], acc[:, :], exp_sb[:, 2 * g + 1, :]
                    )
            state[(h, qb)] = (exp_sb, acc, kmax)

        def stage_pv(h, qb):
            """PV accumulation + normalize-evict into ctxT for (h, qb)."""
            exp_sb, acc, kmax = state.pop((h, qb))
            pv = cpsum.tile([P, FD], F32, tag="pv", bufs=2, name=f"pv{h}_{qb}")
            for kt in range(kmax):
                nc.tensor.matmul(
                    pv[:, :],
                    V[:, kt, h * P : (h + 1) * P],
                    exp_sb[:, kt, :],
                    start=(kt == 0),
                    stop=(kt == kmax - 1),
                )
            # Z: partition-reduce via a single ones-matmul (shares the sg
            # PSUM rotation), approx-reciprocal on DVE, stride-0 DRAM-bounce
            # partition broadcast. Entirely off the PV critical path.
            csum = spsum.tile([1, FD], F32, tag="sg", bufs=2, name=f"cs{h}_{qb}")
            nc.tensor.matmul(
                csum[:, :], ones_sb[:, :], acc[:, :], start=True, stop=True
            )
            rcp = zpool.tile([1, FD], F32, tag="rcp", name=f"rcp{h}_{qb}")
            bc = zpool.tile([P, FD], F32, tag="bc", name=f"bc{h}_{qb}")
            nc.vector.reciprocal_approx_fast(out=rcp[:, :], in_=csum[:, :])
            rd = dpool.tile([1, FD], F32, tag="rd", name=f"rd{h}_{qb}")
            nc.sync.dma_start(out=rd[:, :], in_=rcp[:, :])
            nc.sync.dma_start(out=bc[:, :], in_=_bcast_ap(rd[:, :], P))
            nc.vector.tensor_mul(
                ctxT[:, h, qb * FD : (qb + 1) * FD], pv[:, :], bc[:, :]
            )

        def qk_pass(h, j):
            """QK projection for head h, q/k block j (one PSUM bank each)."""
            if j == 0:
                QTs[h] = qkpool.tile([P, nqb, FD], BF16, tag="QT", name=f"QT{h}")
                KTs[h] = qkpool.tile([P, nqb, FD], BF16, tag="KT", name=f"KT{h}")
            wqs, wks = strips[h]
            qps = jpsum.tile([P, FD], F32, tag="proj", bufs=2, name=f"qp{h}_{j}")
            kps = jpsum.tile([P, FD], F32, tag="proj", bufs=2, name=f"kp{h}_{j}")
            for c in range(nkc):
                nc.tensor.matmul(
                    qps[:, :],
                    wqs[:, c, :],
                    xT[:, c, j * FD : (j + 1) * FD],
                    start=(c == 0),
                    stop=(c == nkc - 1),
                )
                nc.tensor.matmul(
                    kps[:, :],
                    wks[:, c, :],
                    xT[:, c, j * FD : (j + 1) * FD],
                    start=(c == 0),
                    stop=(c == nkc - 1),
                )
            nc.scalar.activation(
                QTs[h][:, j, :],
                qps[:, :],
                mybir.ActivationFunctionType.Identity,
                bias=bqk_sb[:, 0, h : h + 1],
            )
            nc.scalar.activation(
                KTs[h][:, j, :],
                kps[:, :],
                mybir.ActivationFunctionType.Identity,
                bias=bqk_sb[:, 1, h : h + 1],
            )
            if j == nqb - 1:
                strips.pop(h)

        # Slot h: attention(h) stages interleaved with QK projection(h+1).
        # Emission pattern keeps the scalar-engine exp stream one stage ahead
        # of the PV consumer while projection matmuls fill PE wait slots.
        for h in range(nhl):
            if h + 2 < nhl:
                load_strip(h + 2)
            hn = h + 1 if h + 1 < nhl else None
            stage_scores(h, 0)
            if hn is not None:
                qk_pass(hn, 0)
            stage_scores(h, 1)
            stage_pv(h, 0)
            if hn is not None:
                qk_pass(hn, 1)
            stage_scores(h, 2)
            stage_pv(h, 1)
            if hn is not None:
                qk_pass(hn, 2)
            stage_scores(h, 3)
            stage_pv(h, 2)
            if hn is not None:
                qk_pass(hn, 3)
            stage_pv(h, 3)
            if h == nhl - 2:
                # xT dead after QK(h=7)'s last pass: free it and prefetch Wo
                # into the vacated space during the final attention slot.
                es_x.close()
                es_wo = ExitStack()
                wopool = es_wo.enter_context(
                    tc.tile_pool(name="wopool", bufs=1, side="right")
                )
                wo_sb = wopool.tile([P, dhl // P, dm], BF16)
                for i in range(dhl // P):
                    nc.gpsimd.dma_start(
                        out=wo_sb[:, i : i + 1, :], in_=wo_d[:, i : i + 1, :]
                    )
        es_attn.close()

        # ---------------- P4: out = ctx @ Wo (bf16 out) ----------------------
        es_p4 = ExitStack()
        opsum = es_p4.enter_context(tc.tile_pool(name="opsum", bufs=4, space="PSUM"))
        opool = es_p4.enter_context(tc.tile_pool(name="opool", bufs=3, side="right"))
        for st in range(nst):
            for mp in range(dm // (2 * FD)):
                ot = opool.tile([P, 2 * FD], BF16, tag="ot")
                for half in range(2):
                    mc = 2 * mp + half
                    ps = opsum.tile([P, FD], F32, tag="ops", bufs=4)
                    for dc in range(dhl // P):
                        nc.tensor.matmul(
                            ps[:, :],
                            ctxT[:, dc, st * P : (st + 1) * P],
                            wo_sb[:, dc, mc * FD : (mc + 1) * FD],
                            start=(dc == 0),
                            stop=(dc == dhl // P - 1),
                        )
                    nc.scalar.copy(ot[:, half * FD : (half + 1) * FD], ps[:, :])
                nc.sync.dma_start(
                    out=out_d[
                        st * P : (st + 1) * P, mp * 2 * FD : (mp + 1) * 2 * FD
                    ],
                    in_=ot[:, :],
                )
        es_p4.close()
        es_wo.close()
        es_az.close()
        es_strip.close()
        es_top.close()

    # Populate .instr bytes for extended-inst InstISA subclasses (the custom
    # DVE reciprocal) — raw Bass skips this Bacc pass; without it neuronxcc
    # fails codegen with "ISA wrong length".
    mybir.codegen_inst_isa_subclasses(nc)
    if fix_waits:
        _fix_sync_waits(nc)
    return nc


def shard_inputs(x, Wq, bq, Wk, bk, Wv, bv, Wo, bo, seq=S, dm=DM, nhl=NHL, nb=B, g_=G):
    """Host-side sharding: returns per-core input maps (bf16 pre-arranged)."""
    dhl = nhl * P
    nkc = dm // P
    xTs = []
    for b in range(nb):
        xt = np.ascontiguousarray(x[b].T).astype(NP_BF16)  # [dm, seq]
        xTs.append(np.ascontiguousarray(xt.reshape(nkc, P, seq).transpose(1, 0, 2)))
    wqs, wks, wvs, wos, bqks = [], [], [], [], []
    for g in range(g_):
        sl = slice(g * dhl, (g + 1) * dhl)
        wq_s = Wq[:, sl].astype(NP_BF16)
        wk_s = Wk[:, sl].astype(NP_BF16)
        wv_s = Wv[:, sl].astype(NP_BF16)
        wo_s = Wo[sl, :].astype(NP_BF16)
        # wq/wk: [nhl, P, nkc, P] strip-major
        wqs.append(
            np.ascontiguousarray(wq_s.reshape(nkc, P, nhl, P).transpose(2, 1, 0, 3))
        )
        wks.append(
            np.ascontiguousarray(wk_s.reshape(nkc, P, nhl, P).transpose(2, 1, 0, 3))
        )
        wvs.append(np.ascontiguousarray(wv_s.reshape(nkc, P, dhl).transpose(1, 0, 2)))
        wos.append(
            np.ascontiguousarray(wo_s.reshape(dhl // P, P, dm).transpose(1, 0, 2))
        )
        bqk = np.stack(
            [
                np.asarray(bq[sl], np.float32).reshape(nhl, P),
                np.asarray(bk[sl], np.float32).reshape(nhl, P),
            ]
        )  # [2, nhl, P]
        bqks.append(np.ascontiguousarray(bqk.transpose(2, 0, 1)))  # [P, 2, nhl]
    in_maps = []
    for c in range(nb * g_):
        b, g = divmod(c, g_)
        in_maps.append(
            {
                "xT": xTs[b],
                "wq": wqs[g],
                "wk": wks[g],
                "wv": wvs[g],
                "wo": wos[g],
                "bqk": bqks[g],
            }
        )
    return in_maps


_CACHE = {}


def _get_nc():
    if "nc" not in _CACHE:
        _CACHE["nc"] = build_nc()
    return _CACHE["nc"]


def run(inputs, trace=False):
    """Run the SPMD kernel; returns (full_output, BassKernelResults)."""
    inputs = {k: np.asarray(v) for k, v in inputs.items()}
    nc = _get_nc()
    in_maps = shard_inputs(**inputs)
    res = run_bass_kernel_spmd(
        nc, in_maps, core_ids=list(range(NCORES)), trace=trace
    )
    Wo = np.asarray(inputs["Wo"], np.float32)
    const_row = (
        np.asarray(inputs["bv"], np.float32) @ Wo + np.asarray(inputs["bo"], np.float32)
    )
    out = np.empty((B, S, DM), np.float32)
    for b in range(B):
        out[b] = (
            np.asarray(res.results[G * b]["out"], np.float32)
            + np.asarray(res.results[G * b + 1]["out"], np.float32)
            + const_row
        )
    return out, res


def kernel(**inputs):
    out, _ = run(inputs, trace=False)
    return out
